# revision 15
# baseline (speedup 1.0000x reference)
"""MiniMax-M2 sparse MoE block on 8 Trainium2 NeuronCores.

Expert-parallel: 2 of 16 experts per core, dense-over-tokens bf16 GEMMs.
Per core:
  - x [512, 2048] f32 replicated; gate transposed+column-permuted on host so
    the core's 2 local experts sit in columns 0/1 (top-2 is permutation
    invariant).
  - PE-transpose x -> xT (f32 for the router, bf16 for the GEMMs).
  - Router: fp32 matmul xT.T @ gate_wT, sigmoid, top-2 via vector.max,
    combine weights c[t,e] = s*(s>=top2)/(top1+top2).
  - Weights stream HBM->SBUF with f32->bf16 cast during DMA (SWDGE), are
    block-dequantized in one tensor_tensor per tile (scale tile broadcast
    along a step-0 AP), then block-transposed with ONE XBAR DMA-transpose
    per tile into [p, blk, c] layout (out[p,b,c] = in[c, b*128+p]).
  - up/gate: gT/uT [i, t] psums accumulated over 16 h-blocks;
    aT = (sigmoid(gT)*gT)*uT*combine (combine broadcast across partitions).
  - down: yT [h, t] psums accumulated over 6 i-blocks (w2T slices as lhsT,
    aT as rhs); plain-copy evacuated; output stored transposed.
  - Output y [2, 2048, 512] f32 per core (one slab per local expert,
    transposed); host sums 16 slabs and transposes once.
"""

import os
import sys
import numpy as np

for _p in ("/opt/trn_rl_repo", "/root/.axon_site/_ro/trn_rl_repo"):
    if os.path.isdir(_p) and _p not in sys.path:
        sys.path.insert(0, _p)
        break

T, H, I, E = 512, 2048, 768, 16
NCORES, EPC = 8, 2
P = 128
HB, IB, TC = H // P, I // P, T // P      # 16, 6, 4

_CACHE = {}


def _emit_loads_only(nc, mybir, pools, dram, e, stages):
    """Ablation: emit just the weight-load (and optionally dequant+transpose)
    pipeline for expert e, no compute."""
    f32 = mybir.dt.float32
    bf16 = mybir.dt.bfloat16
    OP = mybir.AluOpType
    (const, xpool, xt32p, xtbfp, rpool, w13p, w2p, natp, scp, atp, sgp, stp,
     cbp, ps) = pools
    (x_d, gwt_d, w1_d, w3_d, w2_d, w1s_d, w3s_d, w2s_d, y_d) = dram
    for it in range(IB):
        for (wd, wsd) in ((w1_d, w1s_d), (w3_d, w3s_d)):
            nat = natp.tile([P, H], bf16, tag="nat13", name="nat13")
            nc.gpsimd.dma_start(nat[:], wd[e, it * P:(it + 1) * P, :])
            sc = scp.tile([P, HB], f32, tag="sc13", name="sc13")
            nc.sync.dma_start(sc[:], wsd[e, it * P:(it + 1) * P, :])
            if stages >= 2:
                nc.vector.tensor_tensor(
                    out=nat[:].rearrange("p (b c) -> p b c", b=HB),
                    in0=nat[:].rearrange("p (b c) -> p b c", b=HB),
                    in1=sc[:].to_broadcast([P, HB, P]),
                    op=OP.mult,
                )
                wT = w13p.tile([P, HB, P], bf16, tag="w13T", name="w13T")
                nc.scalar.dma_start(wT[:], nat[:], transpose=True)
    for ht in range(HB):
        nat2 = natp.tile([P, I], bf16, tag="nat2", name="nat2")
        nc.gpsimd.dma_start(nat2[:], w2_d[e, ht * P:(ht + 1) * P, :])
        sc2 = scp.tile([P, IB], f32, tag="sc2", name="sc2")
        nc.sync.dma_start(sc2[:], w2s_d[e, ht * P:(ht + 1) * P, :])
        if stages >= 2:
            nc.vector.tensor_tensor(
                out=nat2[:].rearrange("p (b c) -> p b c", b=IB),
                in0=nat2[:].rearrange("p (b c) -> p b c", b=IB),
                in1=sc2[:].to_broadcast([P, IB, P]),
                op=OP.mult,
            )
            w2T = w2p.tile([P, IB, P], bf16, tag="w2T", name="w2T")
            nc.scalar.dma_start(w2T[:], nat2[:], transpose=True)


def _emit_body(nc, mybir, pools, dram, ident, stages=3):
    f32 = mybir.dt.float32
    bf16 = mybir.dt.bfloat16
    AF = mybir.ActivationFunctionType
    OP = mybir.AluOpType
    (const, xpool, xt32p, xtbfp, rpool, w13p, w2p, natp, scp, atp, sgp, stp,
     cbp, ps) = pools
    (x_d, gwt_d, w1_d, w3_d, w2_d, w1s_d, w3s_d, w2s_d, y_d) = dram

    # gate weight tiles [128 h, 16 e]
    gwt_sb = []
    for hb in range(HB):
        g = const.tile([P, E], f32, tag="gwt", name="gwt")
        nc.sync.dma_start(g[:], gwt_d[hb * P:(hb + 1) * P, :])
        gwt_sb.append(g)

    # ---- stage A: x load, transpose, router ----
    xt32 = [xt32p.tile([P, T], f32, tag="xt32", name="xt32")
            for _ in range(HB)]
    xtbf = [xtbfp.tile([P, T], bf16, tag="xtbf", name="xtbf")
            for _ in range(HB)]
    xns = []
    for tcc in range(TC):
        xn = xpool.tile([P, H], f32, tag="xn", name="xn")
        nc.sync.dma_start(xn[:], x_d[tcc * P:(tcc + 1) * P, :])
        xns.append(xn)
    for hb in range(HB if stages >= 3 else 0):
        # pack 4 token-chunk transposes into one PSUM bank, evacuate once
        pt = ps.tile([P, 512], f32, tag="big", name="pt")
        for tcc in range(TC):
            nc.tensor.matmul(pt[:, tcc * P:(tcc + 1) * P],
                             xns[tcc][:, hb * P:(hb + 1) * P], ident[:],
                             is_transpose=True,
                             start=(tcc == 0), stop=(tcc == TC - 1))
        nc.scalar.activation(xt32[hb][:], pt[:], AF.Copy)
        nc.vector.tensor_copy(xtbf[hb][:], pt[:])

    # router + combine weights; cw[tcc] is [128, E] f32, local experts in
    # columns 0/1
    cw = []
    for tcc in range(TC if stages >= 3 else 0):
        pr = ps.tile([P, 512], f32, tag="big", name="pr")
        for hb in range(HB):
            nc.tensor.matmul(pr[:, :E],
                             xt32[hb][:, tcc * P:(tcc + 1) * P],
                             gwt_sb[hb][:],
                             start=(hb == 0), stop=(hb == HB - 1))
        scores = rpool.tile([P, E], f32, tag="scores", name="scores")
        nc.scalar.activation(scores[:], pr[:, :E], AF.Sigmoid)
        m8 = rpool.tile([P, 8], f32, tag="m8", name="m8")
        nc.vector.max(m8[:], scores[:])
        den = rpool.tile([P, 1], f32, tag="den", name="den")
        nc.vector.tensor_add(den[:], m8[:, 0:1], m8[:, 1:2])
        rden = rpool.tile([P, 1], f32, tag="rden", name="rden")
        nc.vector.reciprocal(rden[:], den[:])
        c = rpool.tile([P, E], f32, tag="cw", name="cwt")
        # mask = (s >= top2), then * s, then * 1/(top1+top2)
        nc.vector.tensor_scalar(c[:], scores[:], m8[:, 1:2], None,
                                op0=OP.is_ge)
        nc.vector.tensor_mul(c[:], c[:], scores[:])
        nc.vector.tensor_scalar_mul(c[:], c[:], rden[:])
        cw.append(c)

    # ---- stage B: experts ----
    for e in range(EPC):
        if stages < 3:
            _emit_loads_only(nc, mybir, pools, dram, e, stages)
            continue
        # combine weight broadcast tile cb[p, t] = c_e[t] for all p:
        # PE-transpose each [128,1] column chunk to a row, assemble
        # crow [1, T], then DMA-broadcast across partitions.
        crow = cbp.tile([1, T], f32, tag="crow", name="crow")
        for tcc in range(TC):
            pc = ps.tile([P, 512], f32, tag="big", name="pc")
            nc.tensor.transpose(pc[:1, :P], cw[tcc][:, e:e + 1], ident[:])
            nc.scalar.activation(crow[:, tcc * P:(tcc + 1) * P], pc[:1, :P],
                                 AF.Copy)
        cb = cbp.tile([P, T], f32, tag="cb", name="cb")
        nc.gpsimd.partition_broadcast(cb[:], crow[0:1, :])

        # w1/w3 load + dequant + transpose + up/gate, per i-chunk
        aT = []
        for it in range(IB):
            wTb = []
            for (wd, wsd) in ((w1_d, w1s_d), (w3_d, w3s_d)):
                nat = natp.tile([P, H], bf16, tag="nat13", name="nat13")
                nc.gpsimd.dma_start(nat[:], wd[e, it * P:(it + 1) * P, :])
                sc = scp.tile([P, HB], f32, tag="sc13", name="sc13")
                nc.sync.dma_start(sc[:], wsd[e, it * P:(it + 1) * P, :])
                if stages >= 2:
                    nc.vector.tensor_tensor(
                        out=nat[:].rearrange("p (b c) -> p b c", b=HB),
                        in0=nat[:].rearrange("p (b c) -> p b c", b=HB),
                        in1=sc[:].to_broadcast([P, HB, P]),
                        op=OP.mult,
                    )
                    # one-shot block transpose: wT[p,hb,i] = nat[i, hb*128+p]
                    wT = w13p.tile([P, HB, P], bf16, tag="w13T", name="w13T")
                    eng = nc.scalar if (it % 2 == 0) else nc.sync
                    eng.dma_start(wT[:], nat[:], transpose=True)
                    wTb.append(wT)
            if stages < 3:
                continue
            w1T, w3T = wTb

            pg = ps.tile([P, 512], f32, tag="big", name="pg")
            pu = ps.tile([P, 512], f32, tag="big", name="pu")
            for hb in range(HB):
                nc.tensor.matmul(pg[:], w1T[:, hb, :], xtbf[hb][:],
                                 start=(hb == 0), stop=(hb == HB - 1))
                nc.tensor.matmul(pu[:], w3T[:, hb, :], xtbf[hb][:],
                                 start=(hb == 0), stop=(hb == HB - 1))
            sg = sgp.tile([P, T], bf16, tag="sg", name="sg")
            nc.scalar.activation(sg[:], pg[:], AF.Sigmoid)
            xs = sgp.tile([P, T], bf16, tag="xs", name="xs")
            nc.vector.tensor_tensor(out=xs[:], in0=sg[:], in1=pg[:],
                                    op=OP.mult)
            a = atp.tile([P, T], bf16, tag="aT", name="aT")
            nc.vector.tensor_tensor(out=a[:], in0=xs[:], in1=pu[:],
                                    op=OP.mult)
            # fold per-token combine weight in while aT is hot
            nc.vector.tensor_tensor(out=a[:], in0=a[:], in1=cb[:],
                                    op=OP.mult)
            aT.append(a)

        # w2 load + dequant + transpose: w2Tb[ht][p, ib, c] =
        #   w2T(i=ib*128+p, h=ht*128+c)
        w2Tb = []
        for ht in range(HB):
            nat2 = natp.tile([P, I], bf16, tag="nat2", name="nat2")
            nc.gpsimd.dma_start(nat2[:], w2_d[e, ht * P:(ht + 1) * P, :])
            sc2 = scp.tile([P, IB], f32, tag="sc2", name="sc2")
            nc.sync.dma_start(sc2[:], w2s_d[e, ht * P:(ht + 1) * P, :])
            if stages >= 2:
                nc.vector.tensor_tensor(
                    out=nat2[:].rearrange("p (b c) -> p b c", b=IB),
                    in0=nat2[:].rearrange("p (b c) -> p b c", b=IB),
                    in1=sc2[:].to_broadcast([P, IB, P]),
                    op=OP.mult,
                )
                w2T = w2p.tile([P, IB, P], bf16, tag="w2T", name="w2T")
                nc.scalar.dma_start(w2T[:], nat2[:], transpose=True)
                w2Tb.append(w2T)

        if stages < 3:
            continue
        # down proj in outT orientation: yT[h, t], accumulate over i-blocks
        for ht in range(HB):
            py = ps.tile([P, 512], f32, tag="big", name="py")
            for ib in range(IB):
                nc.tensor.matmul(py[:], w2Tb[ht][:, ib, :], aT[ib][:],
                                 start=(ib == 0), stop=(ib == IB - 1))
            st = stp.tile([P, T], f32, tag="stage", name="stage")
            nc.scalar.activation(st[:], py[:], AF.Copy)
            nc.sync.dma_start(y_d[e, ht * P:(ht + 1) * P, :], st[:])


def build_nc(reps=1, stages=3):
    import concourse.bacc as bacc
    import concourse.mybir as mybir
    import concourse.tile as tile
    from concourse.masks import make_identity
    from contextlib import ExitStack

    f32 = mybir.dt.float32

    nc = bacc.Bacc("TRN2", target_bir_lowering=False, debug=False,
                   num_devices=NCORES)

    x_d = nc.dram_tensor("x", [T, H], f32, kind="ExternalInput")
    gwt_d = nc.dram_tensor("gwt", [H, E], f32, kind="ExternalInput")
    w1_d = nc.dram_tensor("w1", [EPC, I, H], f32, kind="ExternalInput")
    w3_d = nc.dram_tensor("w3", [EPC, I, H], f32, kind="ExternalInput")
    w2_d = nc.dram_tensor("w2", [EPC, H, I], f32, kind="ExternalInput")
    w1s_d = nc.dram_tensor("w1s", [EPC, I, HB], f32, kind="ExternalInput")
    w3s_d = nc.dram_tensor("w3s", [EPC, I, HB], f32, kind="ExternalInput")
    w2s_d = nc.dram_tensor("w2s", [EPC, H, IB], f32, kind="ExternalInput")
    y_d = nc.dram_tensor("y", [EPC, H, T], f32, kind="ExternalOutput")
    dram = (x_d, gwt_d, w1_d, w3_d, w2_d, w1s_d, w3s_d, w2s_d, y_d)

    with tile.TileContext(nc) as tc:
        with ExitStack() as ctx:
            pools = (
                ctx.enter_context(tc.tile_pool(name="const", bufs=1)),
                ctx.enter_context(tc.tile_pool(name="x", bufs=4)),
                ctx.enter_context(tc.tile_pool(name="xt32", bufs=HB)),
                ctx.enter_context(tc.tile_pool(name="xtbf", bufs=HB)),
                ctx.enter_context(tc.tile_pool(name="router", bufs=4)),
                ctx.enter_context(tc.tile_pool(name="w13T", bufs=2 * IB)),
                ctx.enter_context(tc.tile_pool(name="w2T", bufs=HB + 2)),
                ctx.enter_context(tc.tile_pool(name="nat", bufs=3)),
                ctx.enter_context(tc.tile_pool(name="scales", bufs=4)),
                ctx.enter_context(tc.tile_pool(name="aT", bufs=IB + 2)),
                ctx.enter_context(tc.tile_pool(name="sg", bufs=2)),
                ctx.enter_context(tc.tile_pool(name="stage", bufs=4)),
                ctx.enter_context(tc.tile_pool(name="cb", bufs=2)),
                ctx.enter_context(tc.tile_pool(name="ps", bufs=8,
                                               space="PSUM")),
            )
            const = pools[0]
            ident = const.tile([P, P], f32)
            make_identity(nc, ident[:])
            for _rep in range(reps):
                _emit_body(nc, mybir, pools, dram, ident, stages)

    nc.compile()
    return nc


def shard_inputs(hidden_states, gate_w, w1, w1_scale, w3, w3_scale,
                 w2, w2_scale):
    x = np.ascontiguousarray(hidden_states.reshape(T, H), dtype=np.float32)
    in_maps = []
    for c in range(NCORES):
        lo = c * EPC
        perm = [lo, lo + 1] + [i for i in range(E) if i not in (lo, lo + 1)]
        gwt = np.ascontiguousarray(gate_w[perm].T, dtype=np.float32)
        in_maps.append({
            "x": x,
            "gwt": gwt,
            "w1": np.ascontiguousarray(w1[lo:lo + EPC], dtype=np.float32),
            "w3": np.ascontiguousarray(w3[lo:lo + EPC], dtype=np.float32),
            "w2": np.ascontiguousarray(w2[lo:lo + EPC], dtype=np.float32),
            "w1s": np.ascontiguousarray(w1_scale[lo:lo + EPC],
                                        dtype=np.float32),
            "w3s": np.ascontiguousarray(w3_scale[lo:lo + EPC],
                                        dtype=np.float32),
            "w2s": np.ascontiguousarray(w2_scale[lo:lo + EPC],
                                        dtype=np.float32),
        })
    return in_maps


def kernel(hidden_states, gate_w, w1, w1_scale, w3, w3_scale, w2, w2_scale,
           top_k):
    assert int(top_k) == 2
    from concourse.bass_utils import run_bass_kernel_spmd

    hidden_states = np.asarray(hidden_states)
    B, S, _ = hidden_states.shape
    if "nc" not in _CACHE:
        _CACHE["nc"] = build_nc()
    nc = _CACHE["nc"]

    in_maps = shard_inputs(np.asarray(hidden_states), np.asarray(gate_w),
                           np.asarray(w1), np.asarray(w1_scale),
                           np.asarray(w3), np.asarray(w3_scale),
                           np.asarray(w2), np.asarray(w2_scale))
    res = run_bass_kernel_spmd(nc, in_maps, list(range(NCORES)))
    yt = np.zeros((H, T), dtype=np.float32)
    for c in range(NCORES):
        part = np.asarray(res.results[c]["y"], dtype=np.float32)
        yt += part[0]
        yt += part[1]
    return np.ascontiguousarray(yt.T).reshape(B, S, H).astype(np.float32)
